# revision 2
# baseline (speedup 1.0000x reference)
"""SSN superpixel forward (ASTSFormer) on 8 Trainium2 cores - v2.

Same window/one-hot/AllReduce framework as v1, restructured for engine
balance (measured on HW: DVE fp32 TT 1.09ns/col, bf16 TT 0.57ns/col packed,
PE chained (12,256) matmuls fp32 ~430ns / bf16 ~110ns, Pool+DVE concurrency
hazardous, stride-0-innermost DVE reads full speed):

 - e-compute reads cell windows via stride-0 views (no 16x expansion pass).
 - pass-1 soft path all-fp32 (bf16 there flips ~900 argmax labels -> fails
   the rel-err gate; measured by CPU sim).  Per-pixel quantity planes are
   built (dc,q)-major; one s16->8 fold on DVE, then the TensorEngine does
   the remaining segment sums + one-hot cell-row scatter directly from the
   folded planes (24 main + 16 wrap matmuls per dr, hidden under DVE).
 - pass-2 hard path: fp32 e/argmax, bf16 masks/prods, PE does the full
   segment reduction (no DVE folds; 48+32 bf16 matmuls per dr).
 - single 64KB AllReduce per pass; recolor via 9 channel-stacked
   copy_predicated with bf16 masks and scalar-engine mean expansion.
"""
import sys

sys.path.insert(0, "/opt/trn_rl_repo")

import numpy as np
import bass_rust
from concourse import bass, bacc, mybir, tile

F32 = mybir.dt.float32
BF16 = mybir.dt.bfloat16
I16 = mybir.dt.int16
Alu = mybir.AluOpType
Act = mybir.ActivationFunctionType

BIG = 1e18
N_CORES = 8
H = W = 1024
C = 3
S = 4096


def mkap(ap, offset, dims):
    c = ap.copy()
    c.offset = offset
    c.ap = bass_rust.VecI64Pair(dims)
    return c


def mkfree(ap, extra_offset, free_dims):
    """Replace the FREE dims of an SBUF/PSUM AP, keeping its partition dim."""
    c = ap.copy()
    orig = [list(p) for p in c.ap]
    c.offset = c.offset + extra_offset
    c.ap = bass_rust.VecI64Pair([orig[0]] + free_dims)
    return c


def build_nc():
    nc = bacc.Bacc("TRN2", target_bir_lowering=False, debug=False,
                   num_devices=N_CORES)

    xs_d = nc.dram_tensor("xs", [C, 192, W], F32, kind="ExternalInput")
    whot_d = nc.dram_tensor("whot", [128, 16], F32, kind="ExternalInput")
    whalo_d = nc.dram_tensor("whalo", [64, 12], F32, kind="ExternalInput")
    maskA_d = nc.dram_tensor("maskA", [12, 192], F32, kind="ExternalInput")
    place_d = nc.dram_tensor("place", [12, 64], F32, kind="ExternalInput")
    placeT_d = nc.dram_tensor("placeT", [64, 12], F32, kind="ExternalInput")
    out_d = nc.dram_tensor("out", [C, 128, W], F32, kind="ExternalOutput")

    with tile.TileContext(nc) as tc:
        with tc.tile_pool(name="pp", bufs=1) as pp, \
             tc.tile_pool(name="ps", bufs=1, space="PSUM") as psp, \
             tc.tile_pool(name="dr", bufs=1, space="DRAM") as dp:

            # ---------------- stage A: loads --------------------------------
            xt = []
            xh = []
            for c in range(C):
                t = pp.tile([128, W], F32, tag=f"x{c}", name=f"x{c}")
                nc.sync.dma_start(out=t[:, :], in_=xs_d[c, 32:160, :])
                xt.append(t)
                h = pp.tile([64, W], F32, tag=f"M{c}", name=f"xh{c}")
                nc.sync.dma_start(out=h[0:32, :], in_=xs_d[c, 0:32, :])
                nc.sync.dma_start(out=h[32:64, :], in_=xs_d[c, 160:192, :])
                xh.append(h)
            whot = pp.tile([128, 16], F32, tag="whot", name="whot")
            nc.sync.dma_start(out=whot[:, :], in_=whot_d[:, :])
            whalo = pp.tile([64, 12], F32, tag="whalo", name="whalo")
            nc.sync.dma_start(out=whalo[:, :], in_=whalo_d[:, :])
            maskA = pp.tile([12, 192], F32, tag="maskA", name="maskA")
            nc.sync.dma_start(out=maskA[:, :], in_=maskA_d[:, :])
            place = pp.tile([12, 64], F32, tag="place", name="place")
            nc.sync.dma_start(out=place[:, :], in_=place_d[:, :])
            placeT = pp.tile([64, 12], F32, tag="placeT", name="placeT")
            nc.sync.dma_start(out=placeT[:, :], in_=placeT_d[:, :])
            whot_bf = pp.tile([128, 16], BF16, tag="whotbf", name="whotbf")
            nc.scalar.copy(whot_bf[:, :], whot[:, :])
            zlh = pp.tile([1, 256], F32, tag="zlh", name="zlh")
            nc.vector.memset(zlh[:, :], 0.0)

            # dummy AllReduce posted early to absorb per-core launch stagger
            dummy_in = dp.tile([64], F32, tag="dummy_in", name="dummy_in")
            nc.sync.dma_start(out=dummy_in[:], in_=whot_d[0:4, 0:16])
            dummy_out = dp.tile([64], F32, tag="dummy_out", name="dummy_out")
            nc.gpsimd.collective_compute(
                "AllReduce", Alu.add, replica_groups=[list(range(N_CORES))],
                ins=[dummy_in[:].opt()], outs=[dummy_out[:].opt()])

            # bf16 copies of x for pass-2 hard sums
            xb = []
            for c in range(C):
                b = pp.tile([128, W], BF16, tag=f"xb{c}", name=f"xb{c}")
                nc.scalar.copy(b[:, :], xt[c][:, :])
                xb.append(b)

            # ---------------- stage B: block means -> cw2/gg ---------------
            P1 = psp.tile([12, 192], F32, tag="P1", name="P1")
            nc.tensor.matmul(P1[:, :], zlh[0:1, 0:12], zlh[0:1, 0:192],
                             start=True, stop=False)
            for c in range(C):
                rs = pp.tile([128, 64], F32, tag="rs", name=f"rs{c}")
                nc.vector.tensor_reduce(
                    rs[:, :], xt[c][:, :].rearrange("p (c s) -> p c s", s=16),
                    axis=mybir.AxisListType.X, op=Alu.add)
                nc.tensor.matmul(P1[:, c * 64:(c + 1) * 64], whot[:, 2:14],
                                 rs[:, :], start=False, stop=False)
                rsh = pp.tile([64, 64], F32, tag="rsh", name=f"rsh{c}")
                nc.vector.tensor_reduce(
                    rsh[:, :], xh[c][:, :].rearrange("p (c s) -> p c s", s=16),
                    axis=mybir.AxisListType.X, op=Alu.add)
                nc.tensor.matmul(P1[:, c * 64:(c + 1) * 64], whalo[:, :],
                                 rsh[:, :], start=False, stop=(c == C - 1))

            cw = pp.tile([12, 192], F32, tag="cw", name="cw")
            nc.vector.scalar_tensor_tensor(cw[:, :], P1[:, :], 1.0 / 256.0,
                                           maskA[:, :], op0=Alu.mult, op1=Alu.add)
            sqA = pp.tile([12, 192], F32, tag="sqA", name="sqA")
            nc.vector.tensor_tensor(sqA[:, :], cw[:, :], cw[:, :], op=Alu.mult)
            ggrow = pp.tile([12, 64], F32, tag="ggrow", name="ggrow")
            nc.vector.tensor_reduce(
                ggrow[:, :], sqA[:, :].rearrange("p (c j) -> p j c", c=3),
                axis=mybir.AxisListType.X, op=Alu.add)
            cw2 = pp.tile([12, 192], F32, tag="cw2", name="cw2")
            nc.vector.tensor_tensor(cw2[:, :], cw[:, :], cw[:, :], op=Alu.add)

            centflat = dp.tile([C * 768], F32, tag="centflat", name="centflat")
            for c in range(C):
                nc.sync.dma_start(out=centflat[c * 768:(c + 1) * 768],
                                  in_=cw2[:, c * 64:(c + 1) * 64])
            ggflat = dp.tile([768], F32, tag="ggflat", name="ggflat")
            nc.sync.dma_start(out=ggflat[:], in_=ggrow[:, :])

            # ---------------- shared helpers -------------------------------
            def load_cellv(cflat, gflat, dr, name):
                """(128, 264) cell window: 3 ch x 66 + gg x 66, 16x row-rep."""
                cv = pp.tile([128, 264], F32, tag="cv", name=name)
                for c in range(C):
                    nc.sync.dma_start(out=cv[:, c * 66:(c + 1) * 66], in_=mkap(
                        cflat[:], c * 768 + (2 + dr) * 64 - 1,
                        [[64, 8], [0, 16], [1, 66]]))
                nc.sync.dma_start(out=cv[:, 198:264], in_=mkap(
                    gflat[:], (2 + dr) * 64 - 1, [[64, 8], [0, 16], [1, 66]]))
                return cv

            def bc3(x1024):
                return mkfree(x1024[:, :], 0, [[0, 3], [1, 1024]])

            def v3(a):
                return a[:, :].rearrange("p (d b) -> p d b", d=3)

            def e_ops(cv, out3072):
                """out = sum_c x_c * cellv_c(dc-shift view) - gg(view)."""
                t = pp.tile([128, 3072], F32, tag="t", name="t")
                t2 = pp.tile([128, 3072], F32, tag="t2", name="t2")
                nc.vector.tensor_tensor(
                    v3(t), bc3(xt[0]),
                    mkfree(cv[:, :], 0, [[1, 3], [1, 64], [0, 16]]), op=Alu.mult)
                nc.vector.tensor_tensor(
                    v3(t2), bc3(xt[1]),
                    mkfree(cv[:, :], 66, [[1, 3], [1, 64], [0, 16]]), op=Alu.mult)
                nc.vector.tensor_tensor(t[:, :], t[:, :], t2[:, :], op=Alu.add)
                nc.vector.tensor_tensor(
                    v3(t2), bc3(xt[2]),
                    mkfree(cv[:, :], 132, [[1, 3], [1, 64], [0, 16]]), op=Alu.mult)
                nc.vector.tensor_tensor(t[:, :], t[:, :], t2[:, :], op=Alu.add)
                nc.vector.tensor_tensor(
                    v3(out3072), t[:, :].rearrange("p (d b) -> p d b", d=3),
                    mkfree(cv[:, :], 198, [[1, 3], [1, 64], [0, 16]]),
                    op=Alu.subtract)

            def scatter_init(Pacc):
                nc.tensor.matmul(Pacc[:, :], zlh[0:1, 0:12], zlh[0:1, 0:256],
                                 start=True, stop=False)

            def scatter_pe(Pacc, rhs_of, lhs32, dr, nmain, last):
                """Segment sums + one-hot cell-row scatter on the PE.

                Pacc col = 4*jc + q (interleaved so dc-shifts stay flat).
                rhs_of(dc, s, jclo, njc) -> (128, njc, 4) jc-major AP.
                lhs32(d): one-hot lhsT for cell-row shift d.
                nmain: number of s slices.  Pacc must be scatter_init'ed.
                """
                for dci, dc in enumerate((-1, 0, 1)):
                    lh = lhs32(dr)
                    for s in range(nmain):
                        if dc == 0:
                            nc.tensor.matmul(
                                mkfree(Pacc[:, :], 0, [[1, 256]]),
                                lh, rhs_of(dc, s, 0, 64),
                                start=False, stop=False)
                        elif dc == -1:
                            nc.tensor.matmul(
                                mkfree(Pacc[:, :], 0, [[1, 252]]),
                                lh, rhs_of(dc, s, 1, 63),
                                start=False, stop=False)
                            nc.tensor.matmul(
                                mkfree(Pacc[:, :], 252, [[1, 4]]),
                                lhs32(dr - 1), rhs_of(dc, s, 0, 1),
                                start=False, stop=False)
                        else:
                            nc.tensor.matmul(
                                mkfree(Pacc[:, :], 4, [[1, 252]]),
                                lh, rhs_of(dc, s, 0, 63),
                                start=False, stop=False)
                            nc.tensor.matmul(
                                mkfree(Pacc[:, :], 0, [[1, 4]]),
                                lhs32(dr + 1), rhs_of(dc, s, 63, 1),
                                start=False,
                                stop=(last and s == nmain - 1))

            def lhsf(d):
                return whot[:, 2 - d:14 - d]

            def lhsb(d):
                return whot_bf[:, 2 - d:14 - d]

            def ar_launch(Pacc, tagsuf):
                a2s = pp.tile([12, 256], F32, tag="a2s", name=f"a2s{tagsuf}")
                nc.scalar.copy(a2s[:, :], Pacc[:, :])
                PG = psp.tile([64, 256], F32, tag="PG", name=f"PG{tagsuf}")
                nc.tensor.matmul(PG[:, :], place[:, :], a2s[:, :],
                                 start=True, stop=True)
                pgs = pp.tile([64, 256], F32, tag="pgs", name=f"pgs{tagsuf}")
                nc.scalar.copy(pgs[:, :], PG[:, :])
                ar_in = dp.tile([16384], F32, tag=f"ar{tagsuf}_in",
                                name=f"ar{tagsuf}_in")
                nc.sync.dma_start(out=ar_in[:], in_=pgs[:, :])
                ar_out = dp.tile([16384], F32, tag=f"ar{tagsuf}_out",
                                 name=f"ar{tagsuf}_out")
                nc.gpsimd.collective_compute(
                    "AllReduce", Alu.add, replica_groups=[list(range(N_CORES))],
                    ins=[ar_in[:].opt()], outs=[ar_out[:].opt()])
                return ar_out

            def ar_combine(ar_out, tagsuf):
                ars = pp.tile([64, 256], F32, tag="ars", name=f"ars{tagsuf}")
                nc.sync.dma_start(out=ars[:, :], in_=ar_out[:])
                CMB = psp.tile([12, 256], F32, tag=f"CMB{tagsuf}",
                               name=f"CMB{tagsuf}")
                nc.tensor.matmul(CMB[:, :], placeT[:, :], ars[:, :],
                                 start=True, stop=True)
                return CMB

            # ---------------- pass 1: soft weights + weighted cell sums ----
            W3 = [pp.tile([128, 3072], F32, tag=f"W3{d}", name=f"W3{d}")
                  for d in range(3)]
            Etmp = pp.tile([128, 3072], F32, tag="gmB", name="Etmp")
            cellv1 = []
            for di, dr in enumerate((-1, 0, 1)):
                cv = load_cellv(centflat, ggflat, dr, f"cv1_{di}")
                cellv1.append(cv)
                e_ops(cv, Etmp)
                nc.scalar.activation(W3[di][:, :], Etmp[:, :], Act.Exp)

            # Z = sum of the 9 w's; rinv = 1/Z (approx, ~18 bits)
            zd = [pp.tile([128, 1024], F32, tag=("t", "t2", "rsh")[d],
                          name=f"zd{d}") for d in range(3)]
            for d in range(3):
                nc.vector.tensor_tensor(zd[d][:, :], W3[d][:, 0:1024],
                                        W3[d][:, 1024:2048], op=Alu.add)
                nc.vector.tensor_tensor(zd[d][:, :], zd[d][:, :],
                                        W3[d][:, 2048:3072], op=Alu.add)
            Z = pp.tile([128, 1024], F32, tag="rs", name="Z")
            nc.vector.tensor_tensor(Z[:, :], zd[0][:, :], zd[1][:, :], op=Alu.add)
            nc.vector.tensor_tensor(Z[:, :], Z[:, :], zd[2][:, :], op=Alu.add)
            rinv = pp.tile([128, 1024], F32, tag="rinv", name="rinv")
            nc.vector.reciprocal_approx_fast(rinv[:, :], Z[:, :])

            # per dr: PP4 planes (dc-major, q=[w~,x0w,x1w,x2w]), fold s16->8,
            # then PE finishes segment sums + scatters into P1acc.
            PP4 = pp.tile([128, 12288], F32, tag="PP4", name="PP4")
            F1 = pp.tile([128, 6144], F32, tag="F1", name="F1")
            P1acc = psp.tile([12, 256], F32, tag="P1acc", name="P1acc")
            scatter_init(P1acc)
            pl = mkfree(PP4[:, :], 0, [[4096, 3], [1, 1024]])  # q0 planes
            for di, dr in enumerate((-1, 0, 1)):
                nc.vector.tensor_tensor(pl, v3(W3[di]), bc3(rinv), op=Alu.mult)
                for c in range(C):
                    nc.vector.tensor_tensor(
                        mkfree(PP4[:, :], (1 + c) * 1024, [[4096, 3], [1, 1024]]),
                        bc3(xt[c]), pl, op=Alu.mult)
                # fold s 16->8
                nc.vector.tensor_tensor(
                    F1[:, :].rearrange("p (a b c) -> p a b c", a=12, c=8),
                    mkfree(PP4[:, :], 0, [[1024, 12], [16, 64], [1, 8]]),
                    mkfree(PP4[:, :], 8, [[1024, 12], [16, 64], [1, 8]]),
                    op=Alu.add)
                # PE: F1 layout (dc,q,jc,s'): dc 2048, q 512, jc 8, s' 1
                def rhs1(dc, s, jclo, njc, _F1=F1):
                    off = (dc + 1) * 2048 + jclo * 8 + s
                    if njc == 1:
                        return mkfree(_F1[:, :], off, [[512, 4]])
                    return mkfree(_F1[:, :], off, [[8, njc], [512, 4]])
                scatter_pe(P1acc, rhs1, lhsf, dr, 8, last=(di == 2))
            ar1 = ar_launch(P1acc, "1")

            # ---------------- bridge: spf / gg2 ----------------------------
            CMB1 = ar_combine(ar1, "1")
            denp = pp.tile([12, 64], F32, tag="denp", name="denp")
            nc.vector.tensor_scalar_add(denp[:, :],
                                        mkfree(CMB1[:, :], 0, [[4, 64]]), 1e-16)
            rinvD = pp.tile([12, 64], F32, tag="rinvD", name="rinvD")
            nc.vector.reciprocal_approx_fast(rinvD[:, :], denp[:, :])
            spf = pp.tile([12, 192], F32, tag="spf", name="spf")
            nc.vector.tensor_tensor(
                spf[:, :].rearrange("p (c j) -> p c j", c=3),
                mkfree(CMB1[:, :], 1, [[1, 3], [4, 64]]),
                mkfree(rinvD[:, :], 0, [[0, 3], [1, 64]]), op=Alu.mult)
            nc.vector.tensor_tensor(spf[:, :], spf[:, :], maskA[:, :], op=Alu.add)
            sq2 = pp.tile([12, 192], F32, tag="sqA", name="sq2")
            nc.vector.tensor_tensor(sq2[:, :], spf[:, :], spf[:, :], op=Alu.mult)
            gg2row = pp.tile([12, 64], F32, tag="ggrow", name="gg2row")
            nc.vector.tensor_reduce(
                gg2row[:, :], sq2[:, :].rearrange("p (c j) -> p j c", c=3),
                axis=mybir.AxisListType.X, op=Alu.add)
            spf2 = pp.tile([12, 192], F32, tag="cw2", name="spf2")
            nc.vector.tensor_tensor(spf2[:, :], spf[:, :], spf[:, :], op=Alu.add)
            centflat2 = dp.tile([C * 768], F32, tag="centflat2", name="centflat2")
            for c in range(C):
                nc.sync.dma_start(out=centflat2[c * 768:(c + 1) * 768],
                                  in_=spf2[:, c * 64:(c + 1) * 64])
            ggflat2 = dp.tile([768], F32, tag="ggflat2", name="ggflat2")
            nc.sync.dma_start(out=ggflat2[:], in_=gg2row[:, :])

            # ---------------- pass 2: e2, argmax masks, hard sums ----------
            for di, dr in enumerate((-1, 0, 1)):
                cv = load_cellv(centflat2, ggflat2, dr, f"cv2_{di}")
                e_ops(cv, W3[di])
            bp = [pp.tile([128, 1024], F32, tag=("t", "t2", "rsh")[d],
                          name=f"bp{d}") for d in range(3)]
            for d in range(3):
                nc.vector.tensor_tensor(bp[d][:, :], W3[d][:, 0:1024],
                                        W3[d][:, 1024:2048], op=Alu.max)
                nc.vector.tensor_tensor(bp[d][:, :], bp[d][:, :],
                                        W3[d][:, 2048:3072], op=Alu.max)
            best = pp.tile([128, 1024], F32, tag="rs", name="best")
            nc.vector.tensor_tensor(best[:, :], bp[0][:, :], bp[1][:, :],
                                    op=Alu.max)
            nc.vector.tensor_tensor(best[:, :], best[:, :], bp[2][:, :],
                                    op=Alu.max)
            M3 = [pp.tile([128, 3072], BF16, tag=f"M{d}", name=f"M3{d}")
                  for d in range(3)]
            for d in range(3):
                nc.vector.tensor_tensor(v3(M3[d]), v3(W3[d]), bc3(best),
                                        op=Alu.is_equal)

            PPb = pp.tile([128, 12288], BF16, tag="PP4", name="PPb")
            P2acc = psp.tile([12, 256], F32, tag="P2acc", name="P2acc")
            scatter_init(P2acc)
            plb = mkfree(PPb[:, :], 0, [[4096, 3], [1, 1024]])
            for di, dr in enumerate((-1, 0, 1)):
                nc.vector.tensor_copy(plb, v3(M3[di]))
                for c in range(C):
                    nc.vector.tensor_tensor(
                        mkfree(PPb[:, :], (1 + c) * 1024, [[4096, 3], [1, 1024]]),
                        mkfree(xb[c][:, :], 0, [[0, 3], [1, 1024]]),
                        v3(M3[di]), op=Alu.mult)
                # PE from PPb directly: (dc,q,jc,s): dc 4096, q 1024, jc 16, s 1
                def rhs2(dc, s, jclo, njc, _P=PPb):
                    off = (dc + 1) * 4096 + jclo * 16 + s
                    if njc == 1:
                        return mkfree(_P[:, :], off, [[1024, 4]])
                    return mkfree(_P[:, :], off, [[16, njc], [1024, 4]])
                scatter_pe(P2acc, rhs2, lhsb, dr, 16, last=(di == 2))
            ar2 = ar_launch(P2acc, "2")

            # ---------------- means + recolor ------------------------------
            CMB2 = ar_combine(ar2, "2")
            mcnt = pp.tile([12, 64], F32, tag="denp", name="mcnt")
            nc.vector.tensor_scalar_max(mcnt[:, :],
                                        mkfree(CMB2[:, :], 0, [[4, 64]]), 1.0)
            rinvM = pp.tile([12, 64], F32, tag="rinvD", name="rinvM")
            nc.vector.reciprocal_approx_fast(rinvM[:, :], mcnt[:, :])
            means = pp.tile([12, 192], F32, tag="spf", name="means")
            nc.vector.tensor_tensor(
                means[:, :].rearrange("p (c j) -> p c j", c=3),
                mkfree(CMB2[:, :], 1, [[1, 3], [4, 64]]),
                mkfree(rinvM[:, :], 0, [[0, 3], [1, 64]]), op=Alu.mult)
            meansflat = dp.tile([C * 768], F32, tag="meansflat", name="meansflat")
            for c in range(C):
                nc.sync.dma_start(out=meansflat[c * 768:(c + 1) * 768],
                                  in_=means[:, c * 64:(c + 1) * 64])

            out3 = pp.tile([128, 3168], F32, tag="F1", name="out3")
            nc.gpsimd.memset(out3[:, :], 0.0)
            for di, dr in enumerate((-1, 0, 1)):
                cellm = pp.tile([128, 198], F32, tag=f"cm{di}", name=f"cm{di}")
                for c in range(C):
                    nc.sync.dma_start(out=cellm[:, c * 66:(c + 1) * 66], in_=mkap(
                        meansflat[:], c * 768 + (2 + dr) * 64 - 1,
                        [[64, 8], [0, 16], [1, 66]]))
                gmv = pp.tile([128, 3168], F32, tag="gmB", name=f"gm{di}")
                nc.scalar.copy(
                    gmv[:, :].rearrange("p (d j u) -> p d j u", d=3, u=16),
                    mkfree(cellm[:, :], 0, [[66, 3], [1, 66], [0, 16]]))
                for dci, dc in enumerate((-1, 0, 1)):
                    nc.vector.copy_predicated(
                        mkfree(out3[:, :], 0, [[1056, 3], [1, 1024]]),
                        mkfree(M3[di][:, :], dci * 1024,
                               [[0, 3], [1, 1024]]).bitcast(I16),
                        mkfree(gmv[:, :], 16 * dci, [[1056, 3], [1, 1024]]))
            for c in range(C):
                nc.sync.dma_start(out=out_d[c, :, :],
                                  in_=out3[:, c * 1056:c * 1056 + 1024])

    nc.compile()
    return nc


# ---------------- host side ------------------------------------------------

def make_inputs(pixel_features):
    """pixel_features (2,3,1024,1024) -> list of 8 per-core input dicts."""
    img = np.ascontiguousarray(np.asarray(pixel_features, np.float32)[0])
    whot = np.zeros((128, 16), np.float32)
    for q in range(8):
        whot[q * 16:(q + 1) * 16, q + 4] = 1.0
    whalo = np.zeros((64, 12), np.float32)
    for i, m in enumerate((0, 1, 10, 11)):
        whalo[i * 16:(i + 1) * 16, m] = 1.0
    in_maps = []
    for k in range(N_CORES):
        lo, hi = 128 * k - 32, 128 * k + 160
        slab = np.zeros((C, 192, W), np.float32)
        glo, ghi = max(lo, 0), min(hi, H)
        slab[:, glo - lo:ghi - lo, :] = img[:, glo:ghi, :]
        cells = np.arange(512 * k - 128, 512 * k + 640)
        oob = (cells < 0) | (cells >= S)
        maskA = np.where(oob, np.float32(BIG), np.float32(0.0)).reshape(12, 64)
        maskA = np.concatenate([maskA] * 3, axis=1)
        place = np.zeros((12, 64), np.float32)
        for r in range(12):
            m = 8 * k - 2 + r
            if 0 <= m < 64:
                place[r, m] = 1.0
        in_maps.append({
            "xs": slab, "whot": whot, "whalo": whalo,
            "maskA": np.ascontiguousarray(maskA),
            "place": place, "placeT": np.ascontiguousarray(place.T),
        })
    return in_maps


_NC_CACHE = None


def get_nc():
    global _NC_CACHE
    if _NC_CACHE is None:
        _NC_CACHE = build_nc()
    return _NC_CACHE


def kernel(pixel_features):
    from concourse.bass_utils import run_bass_kernel_spmd
    nc = get_nc()
    in_maps = make_inputs(pixel_features)
    res = run_bass_kernel_spmd(nc, in_maps, core_ids=list(range(N_CORES)))
    out = np.empty((1, C, H, W), np.float32)
    for k in range(N_CORES):
        out[0, :, 128 * k:128 * (k + 1), :] = \
            np.asarray(res.results[k]["out"]).reshape(C, 128, W)
    return out


# revision 3
# speedup vs baseline: 1.0407x; 1.0407x over previous
"""SSN superpixel forward (ASTSFormer) on 8 Trainium2 cores — v2.

Same window/one-hot/AllReduce framework as v1, restructured for engine
balance (measured on HW: DVE fp32 TT 1.09ns/col, bf16 TT 0.57ns/col packed,
PE chained (12,256) matmuls fp32 ~430ns / bf16 ~110ns, Pool+DVE concurrency
hazardous, stride-0-innermost DVE reads full speed):

 - e-compute reads cell windows via stride-0 views (no 16x expansion pass).
 - pass-1 soft path all-fp32 (bf16 there flips ~900 argmax labels -> fails
   the rel-err gate; measured by CPU sim).  Per-pixel quantity planes are
   built (dc,q)-major; one s16->8 fold on DVE, then the TensorEngine does
   the remaining segment sums + one-hot cell-row scatter directly from the
   folded planes (24 main + 16 wrap matmuls per dr, hidden under DVE).
 - pass-2 hard path: fp32 e/argmax, bf16 masks/prods, PE does the full
   segment reduction (no DVE folds; 48+32 bf16 matmuls per dr).
 - single 64KB AllReduce per pass; recolor via 9 channel-stacked
   copy_predicated with bf16 masks and scalar-engine mean expansion.
"""
import sys

sys.path.insert(0, "/opt/trn_rl_repo")

import numpy as np
import bass_rust
from concourse import bass, bacc, mybir, tile

F32 = mybir.dt.float32
BF16 = mybir.dt.bfloat16
I16 = mybir.dt.int16
Alu = mybir.AluOpType
Act = mybir.ActivationFunctionType

BIG = 1e18
N_CORES = 8
H = W = 1024
C = 3
S = 4096


def mkap(ap, offset, dims):
    c = ap.copy()
    c.offset = offset
    c.ap = bass_rust.VecI64Pair(dims)
    return c


def mkfree(ap, extra_offset, free_dims):
    """Replace the FREE dims of an SBUF/PSUM AP, keeping its partition dim."""
    c = ap.copy()
    orig = [list(p) for p in c.ap]
    c.offset = c.offset + extra_offset
    c.ap = bass_rust.VecI64Pair([orig[0]] + free_dims)
    return c


def build_nc():
    nc = bacc.Bacc("TRN2", target_bir_lowering=False, debug=False,
                   num_devices=N_CORES)

    xs_d = nc.dram_tensor("xs", [C, 192, W], F32, kind="ExternalInput")
    whot_d = nc.dram_tensor("whot", [128, 16], F32, kind="ExternalInput")
    whalo_d = nc.dram_tensor("whalo", [64, 12], F32, kind="ExternalInput")
    maskA_d = nc.dram_tensor("maskA", [12, 192], F32, kind="ExternalInput")
    place_d = nc.dram_tensor("place", [12, 64], F32, kind="ExternalInput")
    placeT_d = nc.dram_tensor("placeT", [64, 12], F32, kind="ExternalInput")
    out_d = nc.dram_tensor("out", [C, 128, W], F32, kind="ExternalOutput")

    with tile.TileContext(nc) as tc:
        with tc.tile_pool(name="pp", bufs=1) as pp, \
             tc.tile_pool(name="ps", bufs=1, space="PSUM") as psp, \
             tc.tile_pool(name="dr", bufs=1, space="DRAM") as dp:

            # ---------------- stage A: loads --------------------------------
            xt = []
            xh = []
            for c in range(C):
                t = pp.tile([128, W], F32, tag=f"x{c}", name=f"x{c}")
                nc.sync.dma_start(out=t[:, :], in_=xs_d[c, 32:160, :])
                xt.append(t)
                h = pp.tile([64, W], F32, tag=f"M{c}", name=f"xh{c}")
                nc.sync.dma_start(out=h[0:32, :], in_=xs_d[c, 0:32, :])
                nc.sync.dma_start(out=h[32:64, :], in_=xs_d[c, 160:192, :])
                xh.append(h)
            whot = pp.tile([128, 16], F32, tag="whot", name="whot")
            nc.sync.dma_start(out=whot[:, :], in_=whot_d[:, :])
            whalo = pp.tile([64, 12], F32, tag="whalo", name="whalo")
            nc.sync.dma_start(out=whalo[:, :], in_=whalo_d[:, :])
            maskA = pp.tile([12, 192], F32, tag="maskA", name="maskA")
            nc.sync.dma_start(out=maskA[:, :], in_=maskA_d[:, :])
            place = pp.tile([12, 64], F32, tag="place", name="place")
            nc.sync.dma_start(out=place[:, :], in_=place_d[:, :])
            placeT = pp.tile([64, 12], F32, tag="placeT", name="placeT")
            nc.sync.dma_start(out=placeT[:, :], in_=placeT_d[:, :])
            whot_bf = pp.tile([128, 16], BF16, tag="whotbf", name="whotbf")
            nc.scalar.copy(whot_bf[:, :], whot[:, :])
            zlh = pp.tile([1, 256], F32, tag="zlh", name="zlh")
            nc.vector.memset(zlh[:, :], 0.0)

            # dummy AllReduce posted early to absorb per-core launch stagger
            dummy_in = dp.tile([64], F32, tag="dummy_in", name="dummy_in")
            nc.sync.dma_start(out=dummy_in[:], in_=whot_d[0:4, 0:16])
            dummy_out = dp.tile([64], F32, tag="dummy_out", name="dummy_out")
            nc.gpsimd.collective_compute(
                "AllReduce", Alu.add, replica_groups=[list(range(N_CORES))],
                ins=[dummy_in[:].opt()], outs=[dummy_out[:].opt()])

            # bf16 copies of x for pass-2 hard sums
            xb = []
            for c in range(C):
                b = pp.tile([128, W], BF16, tag=f"xb{c}", name=f"xb{c}")
                nc.scalar.copy(b[:, :], xt[c][:, :])
                xb.append(b)

            # ---------------- stage B: block means -> cw2/gg ---------------
            P1 = psp.tile([12, 192], F32, tag="P1", name="P1")
            nc.tensor.matmul(P1[:, :], zlh[0:1, 0:12], zlh[0:1, 0:192],
                             start=True, stop=False)
            for c in range(C):
                rs = pp.tile([128, 64], F32, tag="rs", name=f"rs{c}")
                nc.vector.tensor_reduce(
                    rs[:, :], xt[c][:, :].rearrange("p (c s) -> p c s", s=16),
                    axis=mybir.AxisListType.X, op=Alu.add)
                nc.tensor.matmul(P1[:, c * 64:(c + 1) * 64], whot[:, 2:14],
                                 rs[:, :], start=False, stop=False)
                rsh = pp.tile([64, 64], F32, tag="rsh", name=f"rsh{c}")
                nc.vector.tensor_reduce(
                    rsh[:, :], xh[c][:, :].rearrange("p (c s) -> p c s", s=16),
                    axis=mybir.AxisListType.X, op=Alu.add)
                nc.tensor.matmul(P1[:, c * 64:(c + 1) * 64], whalo[:, :],
                                 rsh[:, :], start=False, stop=(c == C - 1))

            cw = pp.tile([12, 192], F32, tag="cw", name="cw")
            nc.vector.scalar_tensor_tensor(cw[:, :], P1[:, :], 1.0 / 256.0,
                                           maskA[:, :], op0=Alu.mult, op1=Alu.add)
            sqA = pp.tile([12, 192], F32, tag="sqA", name="sqA")
            nc.vector.tensor_tensor(sqA[:, :], cw[:, :], cw[:, :], op=Alu.mult)
            ggrow = pp.tile([12, 64], F32, tag="ggrow", name="ggrow")
            nc.vector.tensor_reduce(
                ggrow[:, :], sqA[:, :].rearrange("p (c j) -> p j c", c=3),
                axis=mybir.AxisListType.X, op=Alu.add)
            cw2 = pp.tile([12, 192], F32, tag="cw2", name="cw2")
            nc.vector.tensor_tensor(cw2[:, :], cw[:, :], cw[:, :], op=Alu.add)

            centflat = dp.tile([C * 768], F32, tag="centflat", name="centflat")
            for c in range(C):
                nc.sync.dma_start(out=centflat[c * 768:(c + 1) * 768],
                                  in_=cw2[:, c * 64:(c + 1) * 64])
            ggflat = dp.tile([768], F32, tag="ggflat", name="ggflat")
            nc.sync.dma_start(out=ggflat[:], in_=ggrow[:, :])

            # ---------------- shared helpers -------------------------------
            def load_cellv(cflat, gflat, dr, name):
                """(128, 264) cell window: 3 ch x 66 + gg x 66, 16x row-rep."""
                cv = pp.tile([128, 264], F32, tag="cv", name=name)
                for c in range(C):
                    nc.sync.dma_start(out=cv[:, c * 66:(c + 1) * 66], in_=mkap(
                        cflat[:], c * 768 + (2 + dr) * 64 - 1,
                        [[64, 8], [0, 16], [1, 66]]))
                nc.sync.dma_start(out=cv[:, 198:264], in_=mkap(
                    gflat[:], (2 + dr) * 64 - 1, [[64, 8], [0, 16], [1, 66]]))
                return cv

            def bc3(x1024):
                return mkfree(x1024[:, :], 0, [[0, 3], [1, 1024]])

            def v3(a):
                return a[:, :].rearrange("p (d b) -> p d b", d=3)

            def e_ops(cv, out3072):
                """out = sum_c x_c * cellv_c(dc-shift view) - gg(view)."""
                t = pp.tile([128, 3072], F32, tag="t", name="t")
                t2 = pp.tile([128, 3072], F32, tag="t2", name="t2")
                nc.vector.tensor_tensor(
                    v3(t), bc3(xt[0]),
                    mkfree(cv[:, :], 0, [[1, 3], [1, 64], [0, 16]]), op=Alu.mult)
                nc.vector.tensor_tensor(
                    v3(t2), bc3(xt[1]),
                    mkfree(cv[:, :], 66, [[1, 3], [1, 64], [0, 16]]), op=Alu.mult)
                nc.vector.tensor_tensor(t[:, :], t[:, :], t2[:, :], op=Alu.add)
                nc.vector.tensor_tensor(
                    v3(t2), bc3(xt[2]),
                    mkfree(cv[:, :], 132, [[1, 3], [1, 64], [0, 16]]), op=Alu.mult)
                nc.vector.tensor_tensor(t[:, :], t[:, :], t2[:, :], op=Alu.add)
                nc.vector.tensor_tensor(
                    v3(out3072), t[:, :].rearrange("p (d b) -> p d b", d=3),
                    mkfree(cv[:, :], 198, [[1, 3], [1, 64], [0, 16]]),
                    op=Alu.subtract)

            def scatter_init(Pacc):
                nc.tensor.matmul(Pacc[:, :], zlh[0:1, 0:12], zlh[0:1, 0:256],
                                 start=True, stop=False)

            def fold1(PP, F, dtF):
                """PP (dc,q,jc,s16) planes -> F (dc | s'*256 + jc*4 + q), s8."""
                for dci in range(3):
                    nc.vector.tensor_tensor(
                        mkfree(F[:, :], dci * 2048, [[1, 4], [4, 64], [256, 8]]),
                        mkfree(PP[:, :], dci * 4096, [[1024, 4], [16, 64], [1, 8]]),
                        mkfree(PP[:, :], dci * 4096 + 8,
                               [[1024, 4], [16, 64], [1, 8]]),
                        op=Alu.add)

            def scatter_pe(Pacc, F, lhs32, dr, last):
                """One-hot cell-row scatter from contiguous fold slices.

                F col = dc*2048 + s'*256 + jc*4 + q;  Pacc col = 4*jc + q.
                Mains grouped by stationary, then the two wrap groups.
                """
                for dci, dc in enumerate((-1, 0, 1)):
                    for sp in range(8):
                        off = dci * 2048 + sp * 256
                        if dc == 0:
                            nc.tensor.matmul(
                                mkfree(Pacc[:, :], 0, [[1, 256]]), lhs32(dr),
                                mkfree(F[:, :], off, [[1, 256]]),
                                start=False, stop=False)
                        elif dc == -1:
                            nc.tensor.matmul(
                                mkfree(Pacc[:, :], 0, [[1, 252]]), lhs32(dr),
                                mkfree(F[:, :], off + 4, [[1, 252]]),
                                start=False, stop=False)
                        else:
                            nc.tensor.matmul(
                                mkfree(Pacc[:, :], 4, [[1, 252]]), lhs32(dr),
                                mkfree(F[:, :], off, [[1, 252]]),
                                start=False, stop=False)
                for sp in range(8):
                    nc.tensor.matmul(
                        mkfree(Pacc[:, :], 0, [[1, 4]]), lhs32(dr + 1),
                        mkfree(F[:, :], 2 * 2048 + sp * 256 + 252, [[1, 4]]),
                        start=False, stop=False)
                for sp in range(8):
                    nc.tensor.matmul(
                        mkfree(Pacc[:, :], 252, [[1, 4]]), lhs32(dr - 1),
                        mkfree(F[:, :], 0 * 2048 + sp * 256, [[1, 4]]),
                        start=False, stop=(last and sp == 7))

            def lhsf(d):
                return whot[:, 2 - d:14 - d]

            def lhsb(d):
                return whot_bf[:, 2 - d:14 - d]

            def ar_launch(Pacc, tagsuf):
                a2s = pp.tile([12, 256], F32, tag="a2s", name=f"a2s{tagsuf}")
                nc.scalar.copy(a2s[:, :], Pacc[:, :])
                PG = psp.tile([64, 256], F32, tag="PG", name=f"PG{tagsuf}")
                nc.tensor.matmul(PG[:, :], place[:, :], a2s[:, :],
                                 start=True, stop=True)
                pgs = pp.tile([64, 256], F32, tag="pgs", name=f"pgs{tagsuf}")
                nc.scalar.copy(pgs[:, :], PG[:, :])
                ar_in = dp.tile([16384], F32, tag=f"ar{tagsuf}_in",
                                name=f"ar{tagsuf}_in")
                nc.sync.dma_start(out=ar_in[:], in_=pgs[:, :])
                ar_out = dp.tile([16384], F32, tag=f"ar{tagsuf}_out",
                                 name=f"ar{tagsuf}_out")
                nc.gpsimd.collective_compute(
                    "AllReduce", Alu.add, replica_groups=[list(range(N_CORES))],
                    ins=[ar_in[:].opt()], outs=[ar_out[:].opt()])
                return ar_out

            def ar_combine(ar_out, tagsuf):
                ars = pp.tile([64, 256], F32, tag="ars", name=f"ars{tagsuf}")
                nc.sync.dma_start(out=ars[:, :], in_=ar_out[:])
                CMB = psp.tile([12, 256], F32, tag=f"CMB{tagsuf}",
                               name=f"CMB{tagsuf}")
                nc.tensor.matmul(CMB[:, :], placeT[:, :], ars[:, :],
                                 start=True, stop=True)
                return CMB

            # ---------------- pass 1: soft weights + weighted cell sums ----
            W3 = [pp.tile([128, 3072], F32, tag=f"W3{d}", name=f"W3{d}")
                  for d in range(3)]
            Etmp = pp.tile([128, 3072], F32, tag="gmB", name="Etmp")
            cellv1 = []
            for di, dr in enumerate((-1, 0, 1)):
                cv = load_cellv(centflat, ggflat, dr, f"cv1_{di}")
                cellv1.append(cv)
                e_ops(cv, Etmp)
                nc.scalar.activation(W3[di][:, :], Etmp[:, :], Act.Exp)

            # Z = sum of the 9 w's; rinv = 1/Z (approx, ~18 bits)
            zd = [pp.tile([128, 1024], F32, tag=("t", "t2", "rsh")[d],
                          name=f"zd{d}") for d in range(3)]
            for d in range(3):
                nc.vector.tensor_tensor(zd[d][:, :], W3[d][:, 0:1024],
                                        W3[d][:, 1024:2048], op=Alu.add)
                nc.vector.tensor_tensor(zd[d][:, :], zd[d][:, :],
                                        W3[d][:, 2048:3072], op=Alu.add)
            Z = pp.tile([128, 1024], F32, tag="rs", name="Z")
            nc.vector.tensor_tensor(Z[:, :], zd[0][:, :], zd[1][:, :], op=Alu.add)
            nc.vector.tensor_tensor(Z[:, :], Z[:, :], zd[2][:, :], op=Alu.add)
            rinv = pp.tile([128, 1024], F32, tag="rinv", name="rinv")
            nc.vector.reciprocal_approx_fast(rinv[:, :], Z[:, :])

            # per dr: PP4 planes (dc-major, q=[w~,x0w,x1w,x2w]), fold s16->8,
            # then PE finishes segment sums + scatters into P1acc.
            PP4 = pp.tile([128, 12288], F32, tag="PP4", name="PP4")
            F1 = pp.tile([128, 6144], F32, tag="F1", name="F1")
            P1acc = psp.tile([12, 256], F32, tag="P1acc", name="P1acc")
            scatter_init(P1acc)
            pl = mkfree(PP4[:, :], 0, [[4096, 3], [1, 1024]])  # q0 planes
            for di, dr in enumerate((-1, 0, 1)):
                nc.vector.tensor_tensor(pl, v3(W3[di]), bc3(rinv), op=Alu.mult)
                for c in range(C):
                    nc.vector.tensor_tensor(
                        mkfree(PP4[:, :], (1 + c) * 1024, [[4096, 3], [1, 1024]]),
                        bc3(xt[c]), pl, op=Alu.mult)
                fold1(PP4, F1, F32)
                scatter_pe(P1acc, F1, lhsf, dr, last=(di == 2))
            ar1 = ar_launch(P1acc, "1")

            # ---------------- bridge: spf / gg2 ----------------------------
            CMB1 = ar_combine(ar1, "1")
            denp = pp.tile([12, 64], F32, tag="denp", name="denp")
            nc.vector.tensor_scalar_add(denp[:, :],
                                        mkfree(CMB1[:, :], 0, [[4, 64]]), 1e-16)
            rinvD = pp.tile([12, 64], F32, tag="rinvD", name="rinvD")
            nc.vector.reciprocal_approx_fast(rinvD[:, :], denp[:, :])
            spf = pp.tile([12, 192], F32, tag="spf", name="spf")
            nc.vector.tensor_tensor(
                spf[:, :].rearrange("p (c j) -> p c j", c=3),
                mkfree(CMB1[:, :], 1, [[1, 3], [4, 64]]),
                mkfree(rinvD[:, :], 0, [[0, 3], [1, 64]]), op=Alu.mult)
            nc.vector.tensor_tensor(spf[:, :], spf[:, :], maskA[:, :], op=Alu.add)
            sq2 = pp.tile([12, 192], F32, tag="sqA", name="sq2")
            nc.vector.tensor_tensor(sq2[:, :], spf[:, :], spf[:, :], op=Alu.mult)
            gg2row = pp.tile([12, 64], F32, tag="ggrow", name="gg2row")
            nc.vector.tensor_reduce(
                gg2row[:, :], sq2[:, :].rearrange("p (c j) -> p j c", c=3),
                axis=mybir.AxisListType.X, op=Alu.add)
            spf2 = pp.tile([12, 192], F32, tag="cw2", name="spf2")
            nc.vector.tensor_tensor(spf2[:, :], spf[:, :], spf[:, :], op=Alu.add)
            centflat2 = dp.tile([C * 768], F32, tag="centflat2", name="centflat2")
            for c in range(C):
                nc.sync.dma_start(out=centflat2[c * 768:(c + 1) * 768],
                                  in_=spf2[:, c * 64:(c + 1) * 64])
            ggflat2 = dp.tile([768], F32, tag="ggflat2", name="ggflat2")
            nc.sync.dma_start(out=ggflat2[:], in_=gg2row[:, :])

            # ---------------- pass 2: e2, argmax masks, hard sums ----------
            for di, dr in enumerate((-1, 0, 1)):
                cv = load_cellv(centflat2, ggflat2, dr, f"cv2_{di}")
                e_ops(cv, W3[di])
            bp = [pp.tile([128, 1024], F32, tag=("t", "t2", "rsh")[d],
                          name=f"bp{d}") for d in range(3)]
            for d in range(3):
                nc.vector.tensor_tensor(bp[d][:, :], W3[d][:, 0:1024],
                                        W3[d][:, 1024:2048], op=Alu.max)
                nc.vector.tensor_tensor(bp[d][:, :], bp[d][:, :],
                                        W3[d][:, 2048:3072], op=Alu.max)
            best = pp.tile([128, 1024], F32, tag="rs", name="best")
            nc.vector.tensor_tensor(best[:, :], bp[0][:, :], bp[1][:, :],
                                    op=Alu.max)
            nc.vector.tensor_tensor(best[:, :], best[:, :], bp[2][:, :],
                                    op=Alu.max)
            M3 = [pp.tile([128, 3072], BF16, tag=f"M{d}", name=f"M3{d}")
                  for d in range(3)]
            for d in range(3):
                nc.vector.tensor_tensor(v3(M3[d]), v3(W3[d]), bc3(best),
                                        op=Alu.is_equal)

            PPb = pp.tile([128, 12288], BF16, tag="PP4", name="PPb")
            F1b = pp.tile([128, 6144], BF16, tag="F1", name="F1b")
            P2acc = psp.tile([12, 256], F32, tag="P2acc", name="P2acc")
            scatter_init(P2acc)
            plb = mkfree(PPb[:, :], 0, [[4096, 3], [1, 1024]])
            for di, dr in enumerate((-1, 0, 1)):
                nc.vector.tensor_copy(plb, v3(M3[di]))
                for c in range(C):
                    nc.vector.tensor_tensor(
                        mkfree(PPb[:, :], (1 + c) * 1024, [[4096, 3], [1, 1024]]),
                        mkfree(xb[c][:, :], 0, [[0, 3], [1, 1024]]),
                        v3(M3[di]), op=Alu.mult)
                fold1(PPb, F1b, BF16)
                scatter_pe(P2acc, F1b, lhsb, dr, last=(di == 2))
            ar2 = ar_launch(P2acc, "2")

            # ---------------- means + recolor ------------------------------
            CMB2 = ar_combine(ar2, "2")
            mcnt = pp.tile([12, 64], F32, tag="denp", name="mcnt")
            nc.vector.tensor_scalar_max(mcnt[:, :],
                                        mkfree(CMB2[:, :], 0, [[4, 64]]), 1.0)
            rinvM = pp.tile([12, 64], F32, tag="rinvD", name="rinvM")
            nc.vector.reciprocal_approx_fast(rinvM[:, :], mcnt[:, :])
            means = pp.tile([12, 192], F32, tag="spf", name="means")
            nc.vector.tensor_tensor(
                means[:, :].rearrange("p (c j) -> p c j", c=3),
                mkfree(CMB2[:, :], 1, [[1, 3], [4, 64]]),
                mkfree(rinvM[:, :], 0, [[0, 3], [1, 64]]), op=Alu.mult)
            meansflat = dp.tile([C * 768], F32, tag="meansflat", name="meansflat")
            for c in range(C):
                nc.sync.dma_start(out=meansflat[c * 768:(c + 1) * 768],
                                  in_=means[:, c * 64:(c + 1) * 64])

            out3 = pp.tile([128, 3168], F32, tag="F1", name="out3")
            nc.gpsimd.memset(out3[:, :], 0.0)
            for di, dr in enumerate((-1, 0, 1)):
                cellm = pp.tile([128, 198], F32, tag=f"cm{di}", name=f"cm{di}")
                for c in range(C):
                    nc.sync.dma_start(out=cellm[:, c * 66:(c + 1) * 66], in_=mkap(
                        meansflat[:], c * 768 + (2 + dr) * 64 - 1,
                        [[64, 8], [0, 16], [1, 66]]))
                gmv = pp.tile([128, 3168], F32, tag="gmB", name=f"gm{di}")
                nc.scalar.copy(
                    gmv[:, :].rearrange("p (d j u) -> p d j u", d=3, u=16),
                    mkfree(cellm[:, :], 0, [[66, 3], [1, 66], [0, 16]]))
                for dci, dc in enumerate((-1, 0, 1)):
                    nc.vector.copy_predicated(
                        mkfree(out3[:, :], 0, [[1056, 3], [1, 1024]]),
                        mkfree(M3[di][:, :], dci * 1024,
                               [[0, 3], [1, 1024]]).bitcast(I16),
                        mkfree(gmv[:, :], 16 * dci, [[1056, 3], [1, 1024]]))
            for c in range(C):
                nc.sync.dma_start(out=out_d[c, :, :],
                                  in_=out3[:, c * 1056:c * 1056 + 1024])

    nc.compile()
    return nc


# ---------------- host side ------------------------------------------------

def make_inputs(pixel_features):
    """pixel_features (2,3,1024,1024) -> list of 8 per-core input dicts."""
    img = np.ascontiguousarray(np.asarray(pixel_features, np.float32)[0])
    whot = np.zeros((128, 16), np.float32)
    for q in range(8):
        whot[q * 16:(q + 1) * 16, q + 4] = 1.0
    whalo = np.zeros((64, 12), np.float32)
    for i, m in enumerate((0, 1, 10, 11)):
        whalo[i * 16:(i + 1) * 16, m] = 1.0
    in_maps = []
    for k in range(N_CORES):
        lo, hi = 128 * k - 32, 128 * k + 160
        slab = np.zeros((C, 192, W), np.float32)
        glo, ghi = max(lo, 0), min(hi, H)
        slab[:, glo - lo:ghi - lo, :] = img[:, glo:ghi, :]
        cells = np.arange(512 * k - 128, 512 * k + 640)
        oob = (cells < 0) | (cells >= S)
        maskA = np.where(oob, np.float32(BIG), np.float32(0.0)).reshape(12, 64)
        maskA = np.concatenate([maskA] * 3, axis=1)
        place = np.zeros((12, 64), np.float32)
        for r in range(12):
            m = 8 * k - 2 + r
            if 0 <= m < 64:
                place[r, m] = 1.0
        in_maps.append({
            "xs": slab, "whot": whot, "whalo": whalo,
            "maskA": np.ascontiguousarray(maskA),
            "place": place, "placeT": np.ascontiguousarray(place.T),
        })
    return in_maps


_NC_CACHE = None


def get_nc():
    global _NC_CACHE
    if _NC_CACHE is None:
        _NC_CACHE = build_nc()
    return _NC_CACHE


def kernel(pixel_features):
    from concourse.bass_utils import run_bass_kernel_spmd
    nc = get_nc()
    in_maps = make_inputs(pixel_features)
    res = run_bass_kernel_spmd(nc, in_maps, core_ids=list(range(N_CORES)))
    out = np.empty((1, C, H, W), np.float32)
    for k in range(N_CORES):
        out[0, :, 128 * k:128 * (k + 1), :] = \
            np.asarray(res.results[k]["out"]).reshape(C, 128, W)
    return out


# revision 4
# speedup vs baseline: 1.1710x; 1.1252x over previous
"""SSN superpixel forward (ASTSFormer) on 8 Trainium2 cores — v2.

Same window/one-hot/AllReduce framework as v1, restructured for engine
balance (measured on HW: DVE fp32 TT 1.09ns/col, bf16 TT 0.57ns/col packed,
PE chained (12,256) matmuls fp32 ~430ns / bf16 ~110ns, Pool+DVE concurrency
hazardous, stride-0-innermost DVE reads full speed):

 - e-compute reads cell windows via stride-0 views (no 16x expansion pass).
 - pass-1 soft path all-fp32 (bf16 there flips ~900 argmax labels -> fails
   the rel-err gate; measured by CPU sim).  Per-pixel quantity planes are
   built (dc,q)-major; one s16->8 fold on DVE, then the TensorEngine does
   the remaining segment sums + one-hot cell-row scatter directly from the
   folded planes (24 main + 16 wrap matmuls per dr, hidden under DVE).
 - pass-2 hard path: fp32 e/argmax, bf16 masks/prods, PE does the full
   segment reduction (no DVE folds; 48+32 bf16 matmuls per dr).
 - single 64KB AllReduce per pass; recolor via 9 channel-stacked
   copy_predicated with bf16 masks and scalar-engine mean expansion.
"""
import sys

sys.path.insert(0, "/opt/trn_rl_repo")

import numpy as np
import bass_rust
from concourse import bass, bacc, mybir, tile

F32 = mybir.dt.float32
BF16 = mybir.dt.bfloat16
I16 = mybir.dt.int16
Alu = mybir.AluOpType
Act = mybir.ActivationFunctionType

BIG = 1e18
N_CORES = 8
H = W = 1024
C = 3
S = 4096


def mkap(ap, offset, dims):
    c = ap.copy()
    c.offset = offset
    c.ap = bass_rust.VecI64Pair(dims)
    return c


def mkfree(ap, extra_offset, free_dims):
    """Replace the FREE dims of an SBUF/PSUM AP, keeping its partition dim."""
    c = ap.copy()
    orig = [list(p) for p in c.ap]
    c.offset = c.offset + extra_offset
    c.ap = bass_rust.VecI64Pair([orig[0]] + free_dims)
    return c


def build_nc():
    nc = bacc.Bacc("TRN2", target_bir_lowering=False, debug=False,
                   num_devices=N_CORES)

    xs_d = nc.dram_tensor("xs", [C, 192, W], F32, kind="ExternalInput")
    whot_d = nc.dram_tensor("whot", [128, 16], F32, kind="ExternalInput")
    whalo_d = nc.dram_tensor("whalo", [64, 12], F32, kind="ExternalInput")
    maskA_d = nc.dram_tensor("maskA", [12, 192], F32, kind="ExternalInput")
    place_d = nc.dram_tensor("place", [12, 64], F32, kind="ExternalInput")
    placeT_d = nc.dram_tensor("placeT", [64, 12], F32, kind="ExternalInput")
    out_d = nc.dram_tensor("out", [C, 128, W], F32, kind="ExternalOutput")

    with tile.TileContext(nc) as tc:
        with tc.tile_pool(name="pp", bufs=1) as pp, \
             tc.tile_pool(name="ps", bufs=1, space="PSUM") as psp, \
             tc.tile_pool(name="dr", bufs=1, space="DRAM") as dp:

            # ---------------- stage A: loads --------------------------------
            xt = []
            xh = []
            for c in range(C):
                t = pp.tile([128, W], F32, tag=f"x{c}", name=f"x{c}")
                nc.sync.dma_start(out=t[:, :], in_=xs_d[c, 32:160, :])
                xt.append(t)
                h = pp.tile([64, W], F32, tag=f"M{c}", name=f"xh{c}")
                nc.sync.dma_start(out=h[0:32, :], in_=xs_d[c, 0:32, :])
                nc.sync.dma_start(out=h[32:64, :], in_=xs_d[c, 160:192, :])
                xh.append(h)
            whot = pp.tile([128, 16], F32, tag="whot", name="whot")
            nc.sync.dma_start(out=whot[:, :], in_=whot_d[:, :])
            whalo = pp.tile([64, 12], F32, tag="whalo", name="whalo")
            nc.sync.dma_start(out=whalo[:, :], in_=whalo_d[:, :])
            maskA = pp.tile([12, 192], F32, tag="maskA", name="maskA")
            nc.sync.dma_start(out=maskA[:, :], in_=maskA_d[:, :])
            place = pp.tile([12, 64], F32, tag="place", name="place")
            nc.sync.dma_start(out=place[:, :], in_=place_d[:, :])
            placeT = pp.tile([64, 12], F32, tag="placeT", name="placeT")
            nc.sync.dma_start(out=placeT[:, :], in_=placeT_d[:, :])
            whot_bf = pp.tile([128, 16], BF16, tag="whotbf", name="whotbf")
            nc.scalar.copy(whot_bf[:, :], whot[:, :])
            zlh = pp.tile([1, 256], F32, tag="zlh", name="zlh")
            nc.vector.memset(zlh[:, :], 0.0)

            # dummy AllReduce posted early to absorb per-core launch stagger
            dummy_in = dp.tile([64], F32, tag="dummy_in", name="dummy_in")
            nc.sync.dma_start(out=dummy_in[:], in_=whot_d[0:4, 0:16])
            dummy_out = dp.tile([64], F32, tag="dummy_out", name="dummy_out")
            nc.gpsimd.collective_compute(
                "AllReduce", Alu.add, replica_groups=[list(range(N_CORES))],
                ins=[dummy_in[:].opt()], outs=[dummy_out[:].opt()])

            # bf16 copies of x for pass-2 hard sums
            xb = []
            for c in range(C):
                b = pp.tile([128, W], BF16, tag=f"xb{c}", name=f"xb{c}")
                nc.scalar.copy(b[:, :], xt[c][:, :])
                xb.append(b)

            # ---------------- stage B: block means -> cw2/gg ---------------
            P1 = psp.tile([12, 192], F32, tag="P1", name="P1")
            nc.tensor.matmul(P1[:, :], zlh[0:1, 0:12], zlh[0:1, 0:192],
                             start=True, stop=False)
            for c in range(C):
                rs = pp.tile([128, 64], F32, tag="rs", name=f"rs{c}")
                nc.vector.tensor_reduce(
                    rs[:, :], xt[c][:, :].rearrange("p (c s) -> p c s", s=16),
                    axis=mybir.AxisListType.X, op=Alu.add)
                nc.tensor.matmul(P1[:, c * 64:(c + 1) * 64], whot[:, 2:14],
                                 rs[:, :], start=False, stop=False)
                rsh = pp.tile([64, 64], F32, tag="rsh", name=f"rsh{c}")
                nc.vector.tensor_reduce(
                    rsh[:, :], xh[c][:, :].rearrange("p (c s) -> p c s", s=16),
                    axis=mybir.AxisListType.X, op=Alu.add)
                nc.tensor.matmul(P1[:, c * 64:(c + 1) * 64], whalo[:, :],
                                 rsh[:, :], start=False, stop=(c == C - 1))

            cw = pp.tile([12, 192], F32, tag="cw", name="cw")
            nc.vector.scalar_tensor_tensor(cw[:, :], P1[:, :], 1.0 / 256.0,
                                           maskA[:, :], op0=Alu.mult, op1=Alu.add)
            sqA = pp.tile([12, 192], F32, tag="sqA", name="sqA")
            nc.vector.tensor_tensor(sqA[:, :], cw[:, :], cw[:, :], op=Alu.mult)
            ggrow = pp.tile([12, 64], F32, tag="ggrow", name="ggrow")
            nc.vector.tensor_reduce(
                ggrow[:, :], sqA[:, :].rearrange("p (c j) -> p j c", c=3),
                axis=mybir.AxisListType.X, op=Alu.add)
            cw2 = pp.tile([12, 192], F32, tag="cw2", name="cw2")
            nc.vector.tensor_tensor(cw2[:, :], cw[:, :], cw[:, :], op=Alu.add)

            centflat = dp.tile([C * 768], F32, tag="centflat", name="centflat")
            for c in range(C):
                nc.sync.dma_start(out=centflat[c * 768:(c + 1) * 768],
                                  in_=cw2[:, c * 64:(c + 1) * 64])
            ggflat = dp.tile([768], F32, tag="ggflat", name="ggflat")
            nc.sync.dma_start(out=ggflat[:], in_=ggrow[:, :])

            # ---------------- shared helpers -------------------------------
            def load_cellv(cflat, gflat, dr, name):
                """(128, 264) cell window: 3 ch x 66 + gg x 66, 16x row-rep."""
                cv = pp.tile([128, 264], F32, tag="cv", name=name)
                for c in range(C):
                    nc.sync.dma_start(out=cv[:, c * 66:(c + 1) * 66], in_=mkap(
                        cflat[:], c * 768 + (2 + dr) * 64 - 1,
                        [[64, 8], [0, 16], [1, 66]]))
                nc.sync.dma_start(out=cv[:, 198:264], in_=mkap(
                    gflat[:], (2 + dr) * 64 - 1, [[64, 8], [0, 16], [1, 66]]))
                return cv

            def bc3(x1024):
                return mkfree(x1024[:, :], 0, [[0, 3], [1, 1024]])

            def v3(a):
                return a[:, :].rearrange("p (d b) -> p d b", d=3)

            def e_ops(cv, out3072):
                """out = sum_c x_c * cellv_c(dc-shift view) - gg(view)."""
                t = pp.tile([128, 3072], F32, tag="t", name="t")
                t2 = pp.tile([128, 3072], F32, tag="t2", name="t2")
                nc.vector.tensor_tensor(
                    v3(t), bc3(xt[0]),
                    mkfree(cv[:, :], 0, [[1, 3], [1, 64], [0, 16]]), op=Alu.mult)
                nc.vector.tensor_tensor(
                    v3(t2), bc3(xt[1]),
                    mkfree(cv[:, :], 66, [[1, 3], [1, 64], [0, 16]]), op=Alu.mult)
                nc.vector.tensor_tensor(t[:, :], t[:, :], t2[:, :], op=Alu.add)
                nc.vector.tensor_tensor(
                    v3(t2), bc3(xt[2]),
                    mkfree(cv[:, :], 132, [[1, 3], [1, 64], [0, 16]]), op=Alu.mult)
                nc.vector.tensor_tensor(t[:, :], t[:, :], t2[:, :], op=Alu.add)
                nc.vector.tensor_tensor(
                    v3(out3072), t[:, :].rearrange("p (d b) -> p d b", d=3),
                    mkfree(cv[:, :], 198, [[1, 3], [1, 64], [0, 16]]),
                    op=Alu.subtract)

            def scatter_init(Pacc):
                nc.tensor.matmul(Pacc[:, :], zlh[0:1, 0:12], zlh[0:1, 0:256],
                                 start=True, stop=False)

            def fold2(PP, F1, F2):
                """Packed s-folds 16->8->4.  PP (dc,q,jc,s16) -> F2 (dc,q,jc,s4).
                All APs innermost-packed (bf16 tiles get the 2x DVE mode)."""
                nc.vector.tensor_tensor(
                    mkfree(F1[:, :], 0, [[512, 12], [8, 64], [1, 8]]),
                    mkfree(PP[:, :], 0, [[1024, 12], [16, 64], [1, 8]]),
                    mkfree(PP[:, :], 8, [[1024, 12], [16, 64], [1, 8]]),
                    op=Alu.add)
                nc.vector.tensor_tensor(
                    mkfree(F2[:, :], 0, [[256, 12], [4, 64], [1, 4]]),
                    mkfree(F1[:, :], 0, [[512, 12], [8, 64], [1, 4]]),
                    mkfree(F1[:, :], 4, [[512, 12], [8, 64], [1, 4]]),
                    op=Alu.add)

            def scatter_pe(Pacc, F2, lhs32, dr, last):
                """One-hot cell-row scatter; rhs = strided (jc,q) views of F2.

                F2 col = dc*1024 + q*256 + jc*4 + s';  Pacc col = 4*jc + q.
                Mains grouped by stationary, then the two wrap groups.
                """
                for dci, dc in enumerate((-1, 0, 1)):
                    for sp in range(4):
                        off = dci * 1024 + sp
                        if dc == 0:
                            nc.tensor.matmul(
                                mkfree(Pacc[:, :], 0, [[1, 256]]), lhs32(dr),
                                mkfree(F2[:, :], off, [[4, 64], [256, 4]]),
                                start=False, stop=False)
                        elif dc == -1:
                            nc.tensor.matmul(
                                mkfree(Pacc[:, :], 0, [[1, 252]]), lhs32(dr),
                                mkfree(F2[:, :], off + 4, [[4, 63], [256, 4]]),
                                start=False, stop=False)
                        else:
                            nc.tensor.matmul(
                                mkfree(Pacc[:, :], 4, [[1, 252]]), lhs32(dr),
                                mkfree(F2[:, :], off, [[4, 63], [256, 4]]),
                                start=False, stop=False)
                for sp in range(4):
                    nc.tensor.matmul(
                        mkfree(Pacc[:, :], 0, [[1, 4]]), lhs32(dr + 1),
                        mkfree(F2[:, :], 2 * 1024 + 63 * 4 + sp, [[256, 4]]),
                        start=False, stop=False)
                for sp in range(4):
                    nc.tensor.matmul(
                        mkfree(Pacc[:, :], 252, [[1, 4]]), lhs32(dr - 1),
                        mkfree(F2[:, :], 0 * 1024 + sp, [[256, 4]]),
                        start=False, stop=(last and sp == 3))

            def lhsf(d):
                return whot[:, 2 - d:14 - d]

            def lhsb(d):
                return whot_bf[:, 2 - d:14 - d]

            def ar_launch(Pacc, tagsuf):
                a2s = pp.tile([12, 256], F32, tag="a2s", name=f"a2s{tagsuf}")
                nc.scalar.copy(a2s[:, :], Pacc[:, :])
                PG = psp.tile([64, 256], F32, tag="PG", name=f"PG{tagsuf}")
                nc.tensor.matmul(PG[:, :], place[:, :], a2s[:, :],
                                 start=True, stop=True)
                pgs = pp.tile([64, 256], F32, tag="pgs", name=f"pgs{tagsuf}")
                nc.scalar.copy(pgs[:, :], PG[:, :])
                ar_in = dp.tile([16384], F32, tag=f"ar{tagsuf}_in",
                                name=f"ar{tagsuf}_in")
                nc.sync.dma_start(out=ar_in[:], in_=pgs[:, :])
                ar_out = dp.tile([16384], F32, tag=f"ar{tagsuf}_out",
                                 name=f"ar{tagsuf}_out")
                nc.gpsimd.collective_compute(
                    "AllReduce", Alu.add, replica_groups=[list(range(N_CORES))],
                    ins=[ar_in[:].opt()], outs=[ar_out[:].opt()])
                return ar_out

            def ar_combine(ar_out, tagsuf):
                ars = pp.tile([64, 256], F32, tag="ars", name=f"ars{tagsuf}")
                nc.sync.dma_start(out=ars[:, :], in_=ar_out[:])
                CMB = psp.tile([12, 256], F32, tag=f"CMB{tagsuf}",
                               name=f"CMB{tagsuf}")
                nc.tensor.matmul(CMB[:, :], placeT[:, :], ars[:, :],
                                 start=True, stop=True)
                return CMB

            # ---------------- pass 1: soft weights + weighted cell sums ----
            W3 = [pp.tile([128, 3072], F32, tag=f"W3{d}", name=f"W3{d}")
                  for d in range(3)]
            Etmp = pp.tile([128, 3072], F32, tag="gmB", name="Etmp")
            cellv1 = []
            for di, dr in enumerate((-1, 0, 1)):
                cv = load_cellv(centflat, ggflat, dr, f"cv1_{di}")
                cellv1.append(cv)
                e_ops(cv, Etmp)
                nc.scalar.activation(W3[di][:, :], Etmp[:, :], Act.Exp)

            # Z = sum of the 9 w's; rinv = 1/Z (approx, ~18 bits)
            zd = [pp.tile([128, 1024], F32, tag=("t", "t2", "rsh")[d],
                          name=f"zd{d}") for d in range(3)]
            for d in range(3):
                nc.vector.tensor_tensor(zd[d][:, :], W3[d][:, 0:1024],
                                        W3[d][:, 1024:2048], op=Alu.add)
                nc.vector.tensor_tensor(zd[d][:, :], zd[d][:, :],
                                        W3[d][:, 2048:3072], op=Alu.add)
            Z = pp.tile([128, 1024], F32, tag="rs", name="Z")
            nc.vector.tensor_tensor(Z[:, :], zd[0][:, :], zd[1][:, :], op=Alu.add)
            nc.vector.tensor_tensor(Z[:, :], Z[:, :], zd[2][:, :], op=Alu.add)
            rinv = pp.tile([128, 1024], F32, tag="rinv", name="rinv")
            nc.vector.reciprocal_approx_fast(rinv[:, :], Z[:, :])

            # per dr: PP4 planes (dc-major, q=[w~,x0w,x1w,x2w]), fold s16->8,
            # then PE finishes segment sums + scatters into P1acc.
            PP4 = pp.tile([128, 12288], F32, tag="PP4", name="PP4")
            F1 = pp.tile([128, 6144], F32, tag="F1", name="F1")
            F2 = pp.tile([128, 3072], F32, tag="gmB", name="F2")
            P1acc = psp.tile([12, 256], F32, tag="P1acc", name="P1acc")
            scatter_init(P1acc)
            pl = mkfree(PP4[:, :], 0, [[4096, 3], [1, 1024]])  # q0 planes
            for di, dr in enumerate((-1, 0, 1)):
                nc.vector.tensor_tensor(pl, v3(W3[di]), bc3(rinv), op=Alu.mult)
                for c in range(C):
                    nc.vector.tensor_tensor(
                        mkfree(PP4[:, :], (1 + c) * 1024, [[4096, 3], [1, 1024]]),
                        bc3(xt[c]), pl, op=Alu.mult)
                fold2(PP4, F1, F2)
                scatter_pe(P1acc, F2, lhsf, dr, last=(di == 2))
            ar1 = ar_launch(P1acc, "1")

            # ---------------- bridge: spf / gg2 ----------------------------
            CMB1 = ar_combine(ar1, "1")
            denp = pp.tile([12, 64], F32, tag="denp", name="denp")
            nc.vector.tensor_scalar_add(denp[:, :],
                                        mkfree(CMB1[:, :], 0, [[4, 64]]), 1e-16)
            rinvD = pp.tile([12, 64], F32, tag="rinvD", name="rinvD")
            nc.vector.reciprocal_approx_fast(rinvD[:, :], denp[:, :])
            spf = pp.tile([12, 192], F32, tag="spf", name="spf")
            nc.vector.tensor_tensor(
                spf[:, :].rearrange("p (c j) -> p c j", c=3),
                mkfree(CMB1[:, :], 1, [[1, 3], [4, 64]]),
                mkfree(rinvD[:, :], 0, [[0, 3], [1, 64]]), op=Alu.mult)
            nc.vector.tensor_tensor(spf[:, :], spf[:, :], maskA[:, :], op=Alu.add)
            sq2 = pp.tile([12, 192], F32, tag="sqA", name="sq2")
            nc.vector.tensor_tensor(sq2[:, :], spf[:, :], spf[:, :], op=Alu.mult)
            gg2row = pp.tile([12, 64], F32, tag="ggrow", name="gg2row")
            nc.vector.tensor_reduce(
                gg2row[:, :], sq2[:, :].rearrange("p (c j) -> p j c", c=3),
                axis=mybir.AxisListType.X, op=Alu.add)
            spf2 = pp.tile([12, 192], F32, tag="cw2", name="spf2")
            nc.vector.tensor_tensor(spf2[:, :], spf[:, :], spf[:, :], op=Alu.add)
            centflat2 = dp.tile([C * 768], F32, tag="centflat2", name="centflat2")
            for c in range(C):
                nc.sync.dma_start(out=centflat2[c * 768:(c + 1) * 768],
                                  in_=spf2[:, c * 64:(c + 1) * 64])
            ggflat2 = dp.tile([768], F32, tag="ggflat2", name="ggflat2")
            nc.sync.dma_start(out=ggflat2[:], in_=gg2row[:, :])

            # ---------------- pass 2: e2, argmax masks, hard sums ----------
            for di, dr in enumerate((-1, 0, 1)):
                cv = load_cellv(centflat2, ggflat2, dr, f"cv2_{di}")
                e_ops(cv, W3[di])
            bp = [pp.tile([128, 1024], F32, tag=("t", "t2", "rsh")[d],
                          name=f"bp{d}") for d in range(3)]
            for d in range(3):
                nc.vector.tensor_tensor(bp[d][:, :], W3[d][:, 0:1024],
                                        W3[d][:, 1024:2048], op=Alu.max)
                nc.vector.tensor_tensor(bp[d][:, :], bp[d][:, :],
                                        W3[d][:, 2048:3072], op=Alu.max)
            best = pp.tile([128, 1024], F32, tag="rs", name="best")
            nc.vector.tensor_tensor(best[:, :], bp[0][:, :], bp[1][:, :],
                                    op=Alu.max)
            nc.vector.tensor_tensor(best[:, :], best[:, :], bp[2][:, :],
                                    op=Alu.max)
            M3 = [pp.tile([128, 3072], BF16, tag=f"M{d}", name=f"M3{d}")
                  for d in range(3)]
            for d in range(3):
                nc.vector.tensor_tensor(v3(M3[d]), v3(W3[d]), bc3(best),
                                        op=Alu.is_equal)

            PPb = pp.tile([128, 12288], BF16, tag="PP4", name="PPb")
            F1b = pp.tile([128, 6144], BF16, tag="F1", name="F1b")
            F2b = pp.tile([128, 3072], BF16, tag="gmB", name="F2b")
            P2acc = psp.tile([12, 256], F32, tag="P2acc", name="P2acc")
            scatter_init(P2acc)
            plb = mkfree(PPb[:, :], 0, [[4096, 3], [1, 1024]])
            for di, dr in enumerate((-1, 0, 1)):
                nc.vector.tensor_copy(plb, v3(M3[di]))
                for c in range(C):
                    nc.vector.tensor_tensor(
                        mkfree(PPb[:, :], (1 + c) * 1024, [[4096, 3], [1, 1024]]),
                        mkfree(xb[c][:, :], 0, [[0, 3], [1, 1024]]),
                        v3(M3[di]), op=Alu.mult)
                fold2(PPb, F1b, F2b)
                scatter_pe(P2acc, F2b, lhsb, dr, last=(di == 2))
            ar2 = ar_launch(P2acc, "2")

            # ---------------- means + recolor ------------------------------
            CMB2 = ar_combine(ar2, "2")
            mcnt = pp.tile([12, 64], F32, tag="denp", name="mcnt")
            nc.vector.tensor_scalar_max(mcnt[:, :],
                                        mkfree(CMB2[:, :], 0, [[4, 64]]), 1.0)
            rinvM = pp.tile([12, 64], F32, tag="rinvD", name="rinvM")
            nc.vector.reciprocal_approx_fast(rinvM[:, :], mcnt[:, :])
            means = pp.tile([12, 192], F32, tag="spf", name="means")
            nc.vector.tensor_tensor(
                means[:, :].rearrange("p (c j) -> p c j", c=3),
                mkfree(CMB2[:, :], 1, [[1, 3], [4, 64]]),
                mkfree(rinvM[:, :], 0, [[0, 3], [1, 64]]), op=Alu.mult)
            meansflat = dp.tile([C * 768], F32, tag="meansflat", name="meansflat")
            for c in range(C):
                nc.sync.dma_start(out=meansflat[c * 768:(c + 1) * 768],
                                  in_=means[:, c * 64:(c + 1) * 64])

            out3 = pp.tile([128, 3168], F32, tag="F1", name="out3")
            nc.gpsimd.memset(out3[:, :], 0.0)
            for di, dr in enumerate((-1, 0, 1)):
                cellm = pp.tile([128, 198], F32, tag=f"cm{di}", name=f"cm{di}")
                for c in range(C):
                    nc.sync.dma_start(out=cellm[:, c * 66:(c + 1) * 66], in_=mkap(
                        meansflat[:], c * 768 + (2 + dr) * 64 - 1,
                        [[64, 8], [0, 16], [1, 66]]))
                gmv = pp.tile([128, 3168], F32, tag="gmB", name=f"gm{di}")
                nc.scalar.copy(
                    gmv[:, :].rearrange("p (d j u) -> p d j u", d=3, u=16),
                    mkfree(cellm[:, :], 0, [[66, 3], [1, 66], [0, 16]]))
                for dci, dc in enumerate((-1, 0, 1)):
                    nc.vector.copy_predicated(
                        mkfree(out3[:, :], 0, [[1056, 3], [1, 1024]]),
                        mkfree(M3[di][:, :], dci * 1024,
                               [[0, 3], [1, 1024]]).bitcast(I16),
                        mkfree(gmv[:, :], 16 * dci, [[1056, 3], [1, 1024]]))
            for c in range(C):
                nc.sync.dma_start(out=out_d[c, :, :],
                                  in_=out3[:, c * 1056:c * 1056 + 1024])

    nc.compile()
    return nc


# ---------------- host side ------------------------------------------------

def make_inputs(pixel_features):
    """pixel_features (2,3,1024,1024) -> list of 8 per-core input dicts."""
    img = np.ascontiguousarray(np.asarray(pixel_features, np.float32)[0])
    whot = np.zeros((128, 16), np.float32)
    for q in range(8):
        whot[q * 16:(q + 1) * 16, q + 4] = 1.0
    whalo = np.zeros((64, 12), np.float32)
    for i, m in enumerate((0, 1, 10, 11)):
        whalo[i * 16:(i + 1) * 16, m] = 1.0
    in_maps = []
    for k in range(N_CORES):
        lo, hi = 128 * k - 32, 128 * k + 160
        slab = np.zeros((C, 192, W), np.float32)
        glo, ghi = max(lo, 0), min(hi, H)
        slab[:, glo - lo:ghi - lo, :] = img[:, glo:ghi, :]
        cells = np.arange(512 * k - 128, 512 * k + 640)
        oob = (cells < 0) | (cells >= S)
        maskA = np.where(oob, np.float32(BIG), np.float32(0.0)).reshape(12, 64)
        maskA = np.concatenate([maskA] * 3, axis=1)
        place = np.zeros((12, 64), np.float32)
        for r in range(12):
            m = 8 * k - 2 + r
            if 0 <= m < 64:
                place[r, m] = 1.0
        in_maps.append({
            "xs": slab, "whot": whot, "whalo": whalo,
            "maskA": np.ascontiguousarray(maskA),
            "place": place, "placeT": np.ascontiguousarray(place.T),
        })
    return in_maps


_NC_CACHE = None


def get_nc():
    global _NC_CACHE
    if _NC_CACHE is None:
        _NC_CACHE = build_nc()
    return _NC_CACHE


def kernel(pixel_features):
    from concourse.bass_utils import run_bass_kernel_spmd
    nc = get_nc()
    in_maps = make_inputs(pixel_features)
    res = run_bass_kernel_spmd(nc, in_maps, core_ids=list(range(N_CORES)))
    out = np.empty((1, C, H, W), np.float32)
    for k in range(N_CORES):
        out[0, :, 128 * k:128 * (k + 1), :] = \
            np.asarray(res.results[k]["out"]).reshape(C, 128, W)
    return out
